# revision 49
# baseline (speedup 1.0000x reference)
"""Dilated attention (banded local-window attention) for Trainium2.

Problem: q,k,v [1, 16, 4096, 64] fp32; dilation r=2, window 128 (band |i-j|<=64
within each of the 2 strided subsequences of length 2048 per head).

Sharding: 16 heads x 2 offsets = 32 independent blocks -> 8 cores x 2 heads
(4 blocks). The host-side shard step hands each core its q/k already in
d-major ("transposed") per-offset layout [head, r, d, i] so the device reads
Q^T/K^T with full-bandwidth contiguous descriptors; offset r=0 (block A)
lands on SBUF partitions 0:64 and r=1 (block B) on 64:128, which feeds
row-packed K=64 QK matmuls on the two halves of the PE array.

Per block, queries are tiled in 16 tiles of 128; each tile attends to a
256-key window (two 128-chunks at +-64 around the tile). Scores are computed
transposed (S^T[jj, i]) so the probabilities come out pre-transposed for the
PV matmul (no on-chip transpose of P). Softmax skips the max-subtraction
(scores ~ N(0,1) after the 1/8 scale, exp is safe) and folds the 1/8 scale
into the ScalarE exp. The band mask is a 0/1 bf16 multiply after exp; edge
tiles need no special mask because padded V rows carry a zeroed ones-column
(they contribute 0 to both PV and the row sum). Row sums come from that
ones-column appended to V; out = (P@[V|1])[:,:64] * 1/(P@[V|1])[:,64].

Perf notes (measured on TRN2 via NTFF traces): inputs are cast to bf16 on
the host, halving HBM reads and making the loads straight HW/SW-DGE DMAs;
the DMA engines are per-packet limited, so loads keep multi-KB per-partition
lines, split into a need-first chunk (so the first QK starts ~4 us earlier)
with k/v and q on different queue sets to overlap; the output stays in the
device-natural [p, t, (r d)] layout (2 KB DRAM runs instead of 512 B) and
the host un-permutes; PSUM runs ps_pool bufs=3 (QK two groups ahead of exp)
by pointing the DMA-absorber dummy matmuls at group-0's psum banks. The
steady state is jointly limited by the Activation engine (exp, ~1.1 us per
2-tile group) and DVE (mask+norm); the PE holds ~1.2 GHz (no observable
DVFS ramp), and ~10 us of framework preamble/teardown is fixed.
"""

import sys

for _p in ("/opt/trn_rl_repo", "/opt/trn_rl_repo/concourse"):
    if _p not in sys.path:
        sys.path.insert(0, _p)

import numpy as np

import concourse.bass as bass
import concourse.mybir as mybir
import concourse.tile as tile
from concourse import bacc
from concourse.bass_utils import run_bass_kernel_spmd
from concourse.tile_rust import add_dep_helper


def _absorb(nc_engine_nop, deps, reason):
    """Emit an engine nop depending on `deps` so the engine's vector clock
    observes them there; wait-table-capable nops soak up semaphore waits that
    would otherwise overflow the fixed-slot ISA structs (MM=2, LDW=1, DMA=2).
    """
    ab = nc_engine_nop()
    for d in deps:
        if d is not None:
            add_dep_helper(ab.ins, d.ins, reason=reason)
    return ab

N_CORES = 8
B, H, S, D = 1, 16, 4096, 64
R = 2                      # dilation rate
NSEQ = S // R              # 2048 per-offset sequence length
HALF = 64                  # window//2
NT = NSEQ // 128           # 16 query tiles per block
HPC = H // N_CORES         # heads per core = 2

F32 = mybir.dt.float32
BF16 = mybir.dt.bfloat16


def _issue_loads(tc, pools, qT, kT, v, h):
    """Allocate the head's SBUF tiles and issue its load DMAs.

    Emitted before the mask init so the first k/q DMA hits the queue as
    early as the framework preamble allows.
    """
    nc = tc.nc
    (trans, vpool, ppool, opool, rpool, ps_pool, po_pool) = pools

    # ---- transposed Q/K loads (host supplies d-major bf16 layout) ----
    # partition = (r d): A at 0:64, B at 64:128. kT arrives host-padded with
    # 64 zero cols each end so key-window col slicing never leaves the tile.
    qTs = trans.tile([128, NSEQ], BF16, tag="qTs")
    kTp = trans.tile([128, NSEQ + 128], BF16, tag="kTp")
    qTr = qT[h].rearrange("r d i -> (r d) i")
    kTr = kT[h].rearrange("r d i -> (r d) i")
    # V in host-prepared shifted layout + ones column:
    # vsh[p, t, r, :64] = v[h, 2*(128t - 64 + p) + r, :] (0 outside range),
    # vsh[p, t, r, 64] = 1.0 (row-sum trick, zeroed for padded rows).
    vsh = vpool.tile([128, NT + 1, R, 65], BF16, tag="vsh")
    # Loads: the DMA engines are per-packet limited (one packet per
    # partition line), so what matters is line count and which queue set a
    # load sits on. k rides the Pool/SWDGE queues (Q0) while q rides the
    # Sync/HWDGE queues (Q1) so their transfers overlap instead of queueing
    # FIFO behind each other. Head 0 additionally splits k/q into a
    # need-first chunk (groups 0-2) and a bulk chunk.
    hq, hk, hv = (768, 896, 5) if h == 0 else (NSEQ, NSEQ + 128, NT + 1)
    nc.gpsimd.dma_start(kTp[:, 0:hk], kTr[:, 0:hk])
    nc.sync.dma_start(qTs[:, 0:hq], qTr[:, 0:hq])
    nc.gpsimd.dma_start(vsh[:, 0:hv], v[h, :, 0:hv])
    if h == 0:
        # bulk chunks: v2 drains behind k1+v1 on Q0 while q2+k2 ride Q1, so
        # every bulk chunk lands before its group-3/4 consumers.
        nc.gpsimd.dma_start(vsh[:, hv:], v[h, :, hv:])
        nc.sync.dma_start(qTs[:, hq:], qTr[:, hq:])
        nc.sync.dma_start(kTp[:, hk:], kTr[:, hk:])
    return qTs, kTp, vsh, hq, hk, hv


def _build_head(tc, pools, masks, loads, out, h):
    """Emit instructions for one head (both dilation offsets A=even, B=odd)."""
    nc = tc.nc
    (trans, vpool, ppool, opool, rpool, ps_pool, po_pool) = pools
    m_mid = masks
    qTs, kTp, vsh, hq, hk, hv = loads

    out_sb = opool.tile([128, NT, 128], F32, tag="out_sb")

    # Group 0's score and PV tiles are allocated up front so the absorber
    # dummies below can target them (their columns get overwritten by the
    # real matmuls), freeing a PSUM bank for ps_pool bufs=3. k/q dummies go
    # to ps0 (exp(g0) reads ps0, and exp must not wait on v); v/chunk-2
    # dummies go to po0, whose readers already depend on v via PV.
    ps0 = ps_pool.tile([128, 2, 4, 128], F32, tag="ps")
    po0 = po_pool.tile([128, 2, 2, 65], F32, tag="po")

    # PE-proc absorbers: tiny dummy matmuls, one per load DMA, so the PE
    # sequencer observes each load's semaphore here (each dummy carries <=2
    # waits) and the real matmuls below never combine a DMA wait with their
    # steady-state PSUM-WAW + reader-recycle waits (MM ISA limit is 2).
    # k/q chunk-1 absorbers up front (QK needs them); the v absorber lands
    # after group 0's QK matmuls; chunk-2 absorbers before group 2 (first
    # use is group 3).
    def _dummy(psum_ap, tile_ap):
        nc.tensor.matmul(psum_ap, lhsT=tile_ap, rhs=tile_ap,
                         start=True, stop=True)

    _dummy(ps0[0:1, 0, 0, 0:1], kTp[0:1, 0:1])
    _dummy(ps0[0:1, 0, 0, 1:2], qTs[0:1, 0:1])

    def _v_absorber():
        _dummy(po0[0:1, 0, 0, 0:1], vsh[0:1, 0, 0, 0:1])

    def _chunk2_absorbers():
        # Written into ps0's STORAGE (buffer 0): its last reader exp(g0) is
        # long done by group 3, and QK g3 overwrites every dummy column.
        if h == 0:
            _dummy(ps0[0:1, 0, 0, 2:3], kTp[0:1, hk : hk + 1])
            _dummy(ps0[0:1, 0, 0, 3:4], qTs[0:1, hq : hq + 1])
            _dummy(ps0[0:1, 0, 0, 4:5], vsh[0:1, hv, 0, 0:1])

    exps = []       # per group
    mask_ops = []   # per group, list of 2
    norms = []      # per qtile

    # ---- main loop: groups of 2 query tiles share one exp call ----
    for g in range(NT // 2):
        if g == 3:
            _chunk2_absorbers()
        # scores psum, block-major: bank 0 = block A's 4 segs (t0-lo, t0-hi,
        # t1-lo, t1-hi), bank 1 = block B's. Concurrent matmuls from
        # different PE row groups must not share a PSUM bank.
        # The middle K-chunk serves tile 2g's hi seg AND tile 2g+1's lo seg,
        # so it is one N=256 matmul: 3 matmuls + 3 weight loads per block
        # instead of 4.
        ps = ps0 if g == 0 else ps_pool.tile([128, 2, 4, 128], F32, tag="ps")
        q0 = 256 * g
        for blk, (p0, p1) in enumerate(((0, 64), (64, 128))):
            nc.tensor.matmul(
                ps[:, blk, 0, :],
                lhsT=kTp[p0:p1, q0 : q0 + 128],
                rhs=qTs[p0:p1, q0 : q0 + 128],
                start=True,
                stop=True,
            )
            nc.tensor.matmul(
                ps[:, blk, 1:3, :],
                lhsT=kTp[p0:p1, q0 + 128 : q0 + 256],
                rhs=qTs[p0:p1, q0 : q0 + 256],
                start=True,
                stop=True,
            )
            nc.tensor.matmul(
                ps[:, blk, 3, :],
                lhsT=kTp[p0:p1, q0 + 256 : q0 + 384],
                rhs=qTs[p0:p1, q0 + 128 : q0 + 256],
                start=True,
                stop=True,
            )
        if g == 0:
            _v_absorber()

        # exp((q.k)/8) for both tiles in one ScalarE pass; bf16 out.
        # pt/pm buffers are never recycled (bufs=32 covers all groups), so
        # exp carries exactly one wait ([PE]) -- the ACTIVATE/TT/LDW ISA
        # structs have a single sync-wait slot.
        # The very last group is split per block (2 half-size exp/mask ops,
        # PV in block-major order) so the end-of-kernel tail pipelines:
        # block B's exp overlaps block A's mask+PV instead of serializing.
        last = h == HPC - 1 and g == NT // 2 - 1
        scale = 1.0 / float(D) ** 0.5
        pt = ppool.tile([128, 2, 4, 128], BF16, tag="pt")
        pm = ppool.tile([128, 2, 4, 128], BF16, tag="pm")
        if last:
            for blk in range(R):
                exps.append(nc.scalar.activation(
                    pt[:, blk], ps[:, blk],
                    mybir.ActivationFunctionType.Exp, scale=scale,
                ))
                mask_ops.append([nc.vector.tensor_tensor(
                    pm[:, blk].rearrange("p (j c) i -> p j c i", c=2),
                    pt[:, blk].rearrange("p (j c) i -> p j c i", c=2),
                    m_mid[:, None, :, :].to_broadcast((128, 2, 2, 128)),
                    mybir.AluOpType.mult,
                )])
        else:
            exps.append(nc.scalar.activation(
                pt[:], ps[:], mybir.ActivationFunctionType.Exp, scale=scale
            ))

            # band mask (0/1 multiply): lo segs keep i<=jj, hi segs keep
            # i>=jj. Out-of-range (padded) keys need no special mask: their
            # V rows AND ones-column are host-zeroed, so their exp(0)=1
            # probs contribute nothing to the numerator or the row sum.
            mask_ops.append([
                nc.vector.tensor_tensor(
                    pm[:].rearrange("p b (j c) i -> p (b j) c i", c=2),
                    pt[:].rearrange("p b (j c) i -> p (b j) c i", c=2),
                    m_mid[:, None, :, :].to_broadcast((128, 4, 2, 128)),
                    mybir.AluOpType.mult,
                )
            ])

        # PV + row-sum: po[p, j, blk, :] = P_seg.T @ [V|1]
        po = po0 if g == 0 else po_pool.tile([128, 2, 2, 65], F32, tag="po")
        order = (
            [(j, blk) for blk in range(R) for j in range(2)]
            if last
            else [(j, blk) for j in range(2) for blk in range(R)]
        )
        for j, blk in order:
            t = 2 * g + j
            nc.tensor.matmul(
                po[:, j, blk, :],
                lhsT=pm[:, blk, 2 * j + 0, :],
                rhs=vsh[:, t, blk, :],
                start=True,
                stop=False,
            )
            nc.tensor.matmul(
                po[:, j, blk, :],
                lhsT=pm[:, blk, 2 * j + 1, :],
                rhs=vsh[:, t + 1, blk, :],
                start=False,
                stop=True,
            )
        # normalize both tiles at once: out = po[..., 0:64] / po[..., 64].
        # approx_fast (18-bit) is plenty: denominators are softmax row sums
        # >= 1, and the output gate is 2e-2 absmax-relative.
        rc = rpool.tile([128, 2, 2], F32, tag="rc")
        nc.vector.reciprocal_approx_fast(rc[:], po[:, :, :, 64])
        norms.append(
            nc.vector.tensor_tensor(
                out_sb[:, 2 * g : 2 * g + 2, :].rearrange(
                    "p t (r d) -> p t r d", r=R
                ),
                po[:, :, :, 0:64],
                rc[:, :, :, None].to_broadcast((128, 2, R, D)),
                mybir.AluOpType.mult,
            )
        )

        # Out stays in the device-natural [p, t, (r d)] layout: per
        # partition the DRAM run is 2 KB contiguous (vs 512 B for the
        # interleaved [S, D] layout, which quadruples DMA packet count).
        # The host un-permutes after gather. The final flush is only 2
        # tiles so the post-compute DMA tail stays short.
        flush = {1: (0, 4), 3: (4, 4), 6: (8, 6), 7: (14, 2)}.get(g)
        if flush is not None:
            t0, nt = flush
            nc.sync.dma_start(
                out[h][:, t0 : t0 + nt, :],
                out_sb[:, t0 : t0 + nt, :],
            )


def _build_masks(tc, mpool):
    """One [128, 2(lo|hi), 128] bf16 0/1 mask tile.

    Element [jj, c, i]: lo (c=0) keeps i <= jj, hi (c=1) keeps i >= jj.
    Out-of-range keys at the sequence edges are handled by host-zeroed V
    rows + ones-column, so one mask serves every tile.
    """
    nc = tc.nc
    ge = mybir.AluOpType.is_ge
    m = mpool.tile([128, 2, 128], BF16, tag="m_mid")
    nc.gpsimd.memset(m[:], 1.0)
    # lo: keep jj - i >= 0
    nc.gpsimd.affine_select(
        m[:, 0, :], m[:, 0, :], [[-1, 128]], ge, 0.0,
        base=0, channel_multiplier=1,
    )
    # hi: keep i - jj >= 0
    nc.gpsimd.affine_select(
        m[:, 1, :], m[:, 1, :], [[1, 128]], ge, 0.0,
        base=0, channel_multiplier=-1,
    )
    # DVE-proc absorber: the TensorTensor ISA struct takes a single sync
    # wait, so the first real mask multiply must not combine its exp wait
    # with the one-time Pool mask-init wait. This dummy read makes the DVE
    # clock observe the final (= maximal-tick) Pool init op here.
    mdmy = mpool.tile([1, 2], BF16, tag="mdmy")
    nc.vector.tensor_tensor(
        mdmy[0:1, 0:1], m[0:1, 0, 0:1], m[0:1, 1, 0:1],
        mybir.AluOpType.mult,
    )
    return m


def build_bass():
    nc = bacc.Bacc("TRN2", target_bir_lowering=False, debug=False)
    # Inputs arrive pre-cast to bf16 on the host: halves the HBM read bytes
    # and turns the loads into straight (non-casting) DMAs.
    qT = nc.dram_tensor("qT", [HPC, R, D, NSEQ], BF16, kind="ExternalInput")
    kT = nc.dram_tensor("kT", [HPC, R, D, NSEQ + 128], BF16, kind="ExternalInput")
    v = nc.dram_tensor("v", [HPC, 128, NT + 1, R, 65], BF16, kind="ExternalInput")
    out = nc.dram_tensor("out", [HPC, 128, NT, 128], F32, kind="ExternalOutput")

    with tile.TileContext(nc) as tc:
        with (
            tc.tile_pool(name="mpool", bufs=1) as mpool,
            tc.tile_pool(name="trans", bufs=2) as trans,
            tc.tile_pool(name="vpool", bufs=2) as vpool,
            tc.tile_pool(name="ppool", bufs=2 * (NT // 2)) as ppool,
            tc.tile_pool(name="opool", bufs=2) as opool,
            tc.tile_pool(name="rpool", bufs=8) as rpool,
            tc.tile_pool(name="ps_pool", bufs=3, space="PSUM") as ps_pool,
            tc.tile_pool(name="po_pool", bufs=2, space="PSUM") as po_pool,
        ):
            pools = (trans, vpool, ppool, opool, rpool, ps_pool, po_pool)
            loads0 = _issue_loads(tc, pools, qT[:], kT[:], v[:], 0)
            masks = _build_masks(tc, mpool)
            loads1 = _issue_loads(tc, pools, qT[:], kT[:], v[:], 1)
            for h, loads in ((0, loads0), (1, loads1)):
                _build_head(tc, pools, masks, loads, out[:], h)
    nc.compile()
    return nc


_NC_CACHE = None


def kernel(q: np.ndarray, k: np.ndarray, v: np.ndarray) -> np.ndarray:
    global _NC_CACHE
    if _NC_CACHE is None:
        _NC_CACHE = build_bass()
    nc = _NC_CACHE

    import ml_dtypes

    bf16 = ml_dtypes.bfloat16
    q = np.ascontiguousarray(q, dtype=np.float32)
    k = np.ascontiguousarray(k, dtype=np.float32)
    v = np.ascontiguousarray(v, dtype=np.float32)

    # host-side shard + relayout (cast to bf16): q/k to per-offset d-major
    # [h, r, d, i], k zero-padded by 64 cols each end; v to the shifted
    # window layout [h, p, t, r, 65] with a ones column for the row-sum
    # trick. The device kernel previously cast in the DMA; doing it here
    # halves both the axon upload and the device HBM read.
    qT = np.ascontiguousarray(
        q[0].reshape(H, NSEQ, R, D).transpose(0, 2, 3, 1), dtype=bf16
    )
    kT = np.zeros((H, R, D, NSEQ + 2 * HALF), dtype=bf16)
    kT[:, :, :, HALF : HALF + NSEQ] = k[0].reshape(H, NSEQ, R, D).transpose(0, 2, 3, 1)

    vpad = np.zeros((H, NSEQ + 128, R, D + 1), dtype=bf16)
    vpad[:, HALF : HALF + NSEQ, :, :D] = v[0].reshape(H, NSEQ, R, D)
    # ones-column only for REAL rows: padded keys then contribute 0 to both
    # the PV numerator and the row sum, so no edge masks are needed.
    vpad[:, HALF : HALF + NSEQ, :, D] = 1.0
    # vsh[h, p, t, r, :] = vpad[h, 128t + p, r, :]
    vsh = np.empty((H, 128, NT + 1, R, D + 1), dtype=bf16)
    for t in range(NT + 1):
        vsh[:, :, t] = vpad[:, 128 * t : 128 * t + 128]

    in_maps = []
    for c in range(N_CORES):
        hs = slice(c * HPC, (c + 1) * HPC)
        in_maps.append(
            {
                "qT": np.ascontiguousarray(qT[hs]),
                "kT": np.ascontiguousarray(kT[hs]),
                "v": np.ascontiguousarray(vsh[hs]),
            }
        )

    res = run_bass_kernel_spmd(nc, in_maps, core_ids=list(range(N_CORES)))
    out = np.empty((B, H, S, D), dtype=np.float32)
    for c in range(N_CORES):
        # device layout [h, p, t, (r d)] -> [h, t, p, r, d] -> [h, S, D]
        od = res.results[c]["out"].reshape(HPC, 128, NT, R, D)
        out[0, c * HPC : (c + 1) * HPC] = (
            od.transpose(0, 2, 1, 3, 4).reshape(HPC, S, D)
        )
    return out

